# revision 4
# baseline (speedup 1.0000x reference)
"""Trainium2 Bass kernel for nn_Attention_28269474742408.

Single-layer attention block: qkv projections -> softmax attention ->
layernorm -> output projection, for x [8, 1024, 768] (B=8, N=1024, C=768,
H=12 heads, D=64).

Strategy: data parallel over the batch - one batch element per NeuronCore
(8 cores). Everything on-chip per core; no collectives.

Per-core structure (all channel-major, "T" = [channel, token]):
  - Host pre-transposes x[b] -> xT [768, 1024] and all weights -> W.T so
    projections/attention never need on-device transposes.
  - v is projected token-major [1024, 780]: 12 heads x (64 v-cols + a ones
    column); the ones column makes the PV matmul emit softmax denominators
    for free.  The v projection is emitted in column thirds interleaved
    into the pair-0 attention iterations so it hides under the exp stream.
  - Attention runs an mt-outer loop per head PAIR: the two heads' score
    matmuls (K=64 contraction) are row-tiled into the top/bottom halves of
    the PE array (tile_position via base_partition 0/64) so they run
    concurrently - the pair's scores cost one matmul slot instead of two.
  - softmax skips max-subtraction (scores bounded ~|3|, exp can't
    overflow); exp output feeds flash-style PV accumulation in PSUM.
  - Per-head softmax denominators: reciprocal is computed on a DMA-spread
    [128, 8] layout (DVE reciprocal is ~8 cyc per free-dim element, so a
    [1, 1024] row costs 6.5us but the spread costs ~60ns), then broadcast
    back over 64 partitions via a DRAM bounce.  The PSUM accumulator is
    freed early by copying it to SBUF before the division.
  - q/k projections for pair p+1 are emitted as filler units inside pair
    p's iterations, keeping TensorE dense while ScalarE runs exp.
  - LayerNorm is folded into the output projection: gamma/beta folded into
    Wo/bo on the host; mean/var via ones-matmuls, 4-way column-tiled into
    one PSUM bank (partition offsets 0/32/64/96 run concurrently);
    the -mean*colsum(Wo) + sqrt(var+eps)*bo rank-2 correction rides the
    output matmul as an extra K=2 accumulation; the rsqrt scale is applied
    per-token on eviction (built by DMA-spreading the std row to [128, 8]
    and one DVE reciprocal).
  - A dummy exp at t=0 preloads the activation table under the input DMA.
"""
import numpy as np

import concourse.bacc as bacc
import concourse.bass as bass
import concourse.tile as tile
from concourse import bass_isa
from concourse import mybir
from concourse.bass_utils import run_bass_kernel_spmd

F32 = mybir.dt.float32
F16 = mybir.dt.float16
AF = mybir.ActivationFunctionType
OP = mybir.AluOpType

B, N, C, H, D = 8, 1024, 768, 12, 64
KT = C // 128          # 6 channel tiles
NT = N // 128          # 8 token tiles
NP = H // 2            # 6 head pairs
VW = H * (D + 1)       # 780: v plus per-head ones column
VT = VW // 3           # 260: v projection column third
SCALE = D ** -0.5
EPS = 1e-5


def build_kernel():
    nc = bacc.Bacc("TRN2", target_bir_lowering=False)

    xt_d = nc.dram_tensor("xt", (C, N), F16, kind="ExternalInput")
    wv_d = nc.dram_tensor("wv", (C, VW), F16, kind="ExternalInput")
    wqk_d = nc.dram_tensor("wqk", (C, 2 * C), F16, kind="ExternalInput")
    wo_d = nc.dram_tensor("wo", (C, C), F16, kind="ExternalInput")
    extra_d = nc.dram_tensor("extra", (2, C), F16, kind="ExternalInput")
    bqs_d = nc.dram_tensor("bqs", (C,), F32, kind="ExternalInput")
    bve_d = nc.dram_tensor("bve", (VW,), F16, kind="ExternalInput")
    ones_d = nc.dram_tensor("onesd", (128,), F16, kind="ExternalInput")

    y_d = nc.dram_tensor("y", (N, C), F32, kind="ExternalOutput")
    rscr = nc.dram_tensor("rscr", (H, N), F32)     # internal: Z bounce
    rsci = nc.dram_tensor("rsci", (H, N), F32)     # internal: 1/Z bounce
    ascr = nc.dram_tensor("ascr", (1, N), F16)     # internal: std bounce

    with tile.TileContext(nc) as tc:
        with tc.tile_pool(name="persist", bufs=1) as pp, \
             tc.tile_pool(name="attp", bufs=1) as attp, \
             tc.tile_pool(name="wop", bufs=1) as wop, \
             tc.tile_pool(name="sqp", bufs=1) as sqp:

            # ---- constants (loads emitted after the xT DMA, below) ----
            extra_t = pp.tile([2, C], F16, tag="extra", name="extra")
            bqs_t = [pp.tile([128, 1], F32, tag=f"bqs{m}", name=f"bqs{m}")
                     for m in range(NP)]
            ones_col = pp.tile([128, 1], F16, tag="ones_col", name="ones_col")
            eps_row = pp.tile([1, 1], F32, tag="eps_row", name="eps_row")
            warm_row = pp.tile([1, 1], F32, tag="warm_row", name="warm_row")
            nc.vector.memset(eps_row, EPS)
            # dummy exp: forces the exp act-table load under the input DMA
            nc.scalar.activation(out=warm_row, in_=eps_row, func=AF.Exp)
            # bias for v broadcast across all partitions (applied on evict)
            bve_bc = pp.tile([128, VW], F16, tag="bve_bc", name="bve_bc")

            att = [attp.tile([128, N], F16, tag=f"att{k}", name=f"att{k}")
                   for k in range(KT)]
            f_t = pp.tile([2, N], F16, tag="f_t", name="f_t")
            acol = pp.tile([128, NT], F32, tag="acol", name="acol")

            with tc.tile_pool(name="xtp", bufs=1) as xtp, \
                 tc.tile_pool(name="vp", bufs=1) as vp, \
                 tc.tile_pool(name="wvp", bufs=1) as wvp, \
                 tc.tile_pool(name="pairw", bufs=3) as pairw, \
                 tc.tile_pool(name="qkpair", bufs=3) as qkpair, \
                 tc.tile_pool(name="epool", bufs=4) as epool, \
                 tc.tile_pool(name="cpool", bufs=2) as cpool, \
                 tc.tile_pool(name="zspp", bufs=2) as zspp, \
                 tc.tile_pool(name="rbcp", bufs=2) as rbcp:

                xt = [xtp.tile([128, N], F16, tag=f"xt{k}", name=f"xt{k}")
                      for k in range(KT)]
                for k in range(KT):
                    nc.sync.dma_start(out=xt[k], in_=xt_d[k * 128:(k + 1) * 128, :])
                vt = [vp.tile([128, VW], F16, tag=f"vt{n}", name=f"vt{n}")
                      for n in range(NT)]
                wv_t = [wvp.tile([128, VW], F16, tag=f"wv{k}", name=f"wv{k}")
                        for k in range(KT)]
                wo_t = [wop.tile([128, C], F16, tag=f"wo{k}", name=f"wo{k}")
                        for k in range(KT)]
                sq = [sqp.tile([128, N], F16, tag=f"sq{k}", name=f"sq{k}")
                      for k in range(KT)]

                with tc.tile_pool(name="ps_s", bufs=2, space="PSUM") as ps_s, \
                     tc.tile_pool(name="ps_att", bufs=2, space="PSUM") as ps_att:
                    qk_tiles = {}

                    def load_pair_w(p):
                        pw = [pairw.tile([128, 256], F16, tag=f"pw{k}",
                                         name=f"pw{k}") for k in range(KT)]
                        for k in range(KT):
                            nc.sync.dma_start(
                                out=pw[k],
                                in_=wqk_d[k * 128:(k + 1) * 128,
                                          p * 256:(p + 1) * 256])
                        qt = qkpair.tile([128, N], F16, tag="qtp", name="qtp")
                        kt = qkpair.tile([128, N], F16, tag="ktp", name="ktp")
                        qk_tiles[p] = (pw, qt, kt)

                    def emit_proj_group(p, which, ch):
                        # one q-or-k projection chunk for pair p: [128, 512]
                        pw, qt, kt = qk_tiles[p]
                        off = 0 if which == "q" else 128
                        pg = ps_s.tile([128, 512], F32, tag="sp", name="pg")
                        for k in range(KT):
                            nc.tensor.matmul(
                                out=pg, lhsT=pw[k][:, off:off + 128],
                                rhs=xt[k][:, ch * 512:(ch + 1) * 512],
                                start=(k == 0), stop=(k == KT - 1),
                            )
                        if which == "q":
                            nc.vector.tensor_scalar(
                                out=qt[:, ch * 512:(ch + 1) * 512], in0=pg,
                                scalar1=bqs_t[p], scalar2=None, op0=OP.add,
                            )
                        else:
                            nc.vector.tensor_copy(
                                out=kt[:, ch * 512:(ch + 1) * 512], in_=pg)

                    def emit_v_unit(n, c0):
                        # one v-projection third: vt[n][:, c0:c0+VT]
                        pv = ps_s.tile([128, VT], F32, tag="sp", name="pv")
                        for k in range(KT):
                            nc.tensor.matmul(
                                out=pv,
                                lhsT=xt[k][:, n * 128:(n + 1) * 128],
                                rhs=wv_t[k][:, c0:c0 + VT],
                                start=(k == 0), stop=(k == KT - 1),
                            )
                        nc.vector.tensor_tensor(
                            out=vt[n][:, c0:c0 + VT], in0=pv,
                            in1=bve_bc[:, c0:c0 + VT], op=OP.add)

                    # ---- lead-in: pair-0 q/k projections ----
                    load_pair_w(0)
                    # small consts after the latency-critical xT/pw0 loads
                    nc.sync.dma_start(out=extra_t, in_=extra_d[:, :])
                    for m in range(NP):
                        nc.sync.dma_start(
                            out=bqs_t[m],
                            in_=bqs_d[m * 128:(m + 1) * 128].unsqueeze(1))
                    nc.sync.dma_start(out=ones_col, in_=ones_d[:].unsqueeze(1))
                    nc.sync.dma_start(
                        out=bve_bc,
                        in_=bass.AP(tensor=bve_d[:].tensor, offset=0,
                                    ap=[[0, 128], [1, VW]]))
                    for k in range(KT):
                        nc.sync.dma_start(out=wv_t[k],
                                          in_=wv_d[k * 128:(k + 1) * 128, :])
                    for k in range(KT):
                        nc.sync.dma_start(out=wo_t[k],
                                          in_=wo_d[k * 128:(k + 1) * 128, :])
                    for which, ch in (("q", 0), ("q", 1), ("k", 0), ("k", 1)):
                        emit_proj_group(0, which, ch)

                    # ---- filler queue: one unit per (pair, mt) iteration.
                    # v thirds: A (cols 0-259, pairs 0-1) during pair 0;
                    # B (260-519, pairs 2-3) during pair 1; C (520-779,
                    # pairs 4-5) during pair 2.  q/k projections for pair
                    # p+1 ride the first four iterations of pair p.
                    fillers = {}
                    for mt in range(NT):
                        fillers[(0, mt)] = [lambda n=mt: emit_v_unit(n, 0)]
                        fillers[(1, mt)] = [lambda n=mt: emit_v_unit(n, VT)]
                        fillers[(2, mt)] = [lambda n=mt: emit_v_unit(n, 2 * VT)]
                        fillers[(3, mt)] = []
                        fillers[(4, mt)] = []
                        fillers[(5, mt)] = []
                    for p in range(NP - 1):
                        for i, (which, ch) in enumerate(
                                (("q", 0), ("q", 1), ("k", 0), ("k", 1))):
                            fillers[(p, i)].append(
                                lambda p=p, w=which, c=ch:
                                    emit_proj_group(p + 1, w, c))

                    # ---- attention: mt-outer loop per head pair ----
                    for p in range(NP):
                        if p + 1 < NP:
                            load_pair_w(p + 1)
                        _, qt, kt = qk_tiles[p]
                        pa = [ps_att.tile([65, N], F32, tag="pa", name=f"pa{hh}")
                              for hh in range(2)]
                        for mt in range(NT):
                            # TensorE filler first: its PSUM slot is the one
                            # freed by LAST iteration's exp, so the filler
                            # matmuls start immediately while ScalarE still
                            # chews the previous exp.
                            for fn in fillers.pop((p, mt), ()):
                                fn()
                            s_t = [ps_s.tile([128, N], F32, tag="sp",
                                             name=f"s{hh}") for hh in range(2)]
                            # row-tiled pair scores: head hh uses PE array
                            # rows 64*hh..64*hh+63 - both run concurrently
                            for ch in range(2):
                                for hh in range(2):
                                    hr = hh * 64
                                    nc.tensor.matmul(
                                        out=s_t[hh][:, ch * 512:(ch + 1) * 512],
                                        lhsT=kt[hr:hr + 64,
                                                mt * 128:(mt + 1) * 128],
                                        rhs=qt[hr:hr + 64,
                                               ch * 512:(ch + 1) * 512],
                                        start=True, stop=True,
                                    )
                            e_t = []
                            for hh in range(2):
                                e = epool.tile([128, N], F16, tag="e", name="e")
                                nc.scalar.activation(out=e, in_=s_t[hh],
                                                     func=AF.Exp)
                                e_t.append(e)
                            for hh in range(2):
                                h = 2 * p + hh
                                for ch in range(2):
                                    nc.tensor.matmul(
                                        out=pa[hh][:, ch * 512:(ch + 1) * 512],
                                        lhsT=vt[mt][:, h * 65:(h + 1) * 65],
                                        rhs=e_t[hh][:, ch * 512:(ch + 1) * 512],
                                        start=(mt == 0), stop=(mt == NT - 1),
                                    )
                        # ---- per-head eviction: copy PSUM out early, then
                        # spread-reciprocal of the denominator row ----
                        for hh in range(2):
                            h = 2 * p + hh
                            hr = hh * 64
                            cp = cpool.tile([65, N], F32, tag="cp", name="cp")
                            nc.vector.tensor_copy(out=cp, in_=pa[hh])
                            nc.sync.dma_start(out=rscr[h:h + 1, :],
                                              in_=cp[64:65, :])
                            zsp = zspp.tile([128, NT], F32, tag="zsp",
                                            name="zsp")
                            src = rscr[h:h + 1, :]
                            nc.sync.dma_start(
                                out=zsp,
                                in_=bass.AP(tensor=src.tensor, offset=src.offset,
                                            ap=[[1, 128], [128, NT]]))
                            zspi = zspp.tile([128, NT], F32, tag="zspi",
                                             name="zspi")
                            nc.vector.reciprocal(out=zspi, in_=zsp)
                            dst = rsci[h:h + 1, :]
                            nc.sync.dma_start(
                                out=bass.AP(tensor=dst.tensor, offset=dst.offset,
                                            ap=[[1, 128], [128, NT]]),
                                in_=zspi)
                            rbc = rbcp.tile([64, N], F32, tag="rbc", name="rbc")
                            nc.sync.dma_start(
                                out=rbc,
                                in_=bass.AP(tensor=dst.tensor, offset=dst.offset,
                                            ap=[[0, 64], [1, N]]))
                            nc.vector.tensor_tensor(
                                out=att[p][hr:hr + 64, :], in0=cp[0:64, :],
                                in1=rbc, op=OP.mult)
                        nc.vector.tensor_tensor(out=sq[p], in0=att[p],
                                                in1=att[p], op=OP.mult)

            # ---- phase C: LN stats + output projection ----
            with tc.tile_pool(name="rowpool", bufs=1) as rowpool, \
                 tc.tile_pool(name="ypool", bufs=3) as ypool:

                with tc.tile_pool(name="ps_row", bufs=1, space="PSUM") as ps_row:
                    # 4-way column-tiled stats in one PSUM bank: rows
                    # 0/32/64/96 hold sum(att) ch0/ch1, sum(att^2) ch0/ch1
                    rows = ps_row.tile([97, 512], F32, tag="rows", name="rows")
                    for k in range(KT):
                        for j, (srcs, ch) in enumerate(
                                ((att, 0), (att, 1), (sq, 0), (sq, 1))):
                            nc.tensor.matmul(
                                out=rows[32 * j:32 * j + 1, :],
                                lhsT=ones_col,
                                rhs=srcs[k][:, ch * 512:(ch + 1) * 512],
                                start=(k == 0), stop=(k == KT - 1),
                                tile_position=(0, 32 * j),
                            )
                    mrow = rowpool.tile([1, N], F32, tag="mrow", name="mrow")
                    t0 = rowpool.tile([1, N], F32, tag="t0", name="t0")
                    for ch in range(2):
                        sl = slice(ch * 512, (ch + 1) * 512)
                        nc.scalar.mul(out=mrow[:, sl],
                                      in_=rows[32 * ch:32 * ch + 1, :],
                                      mul=1.0 / C)
                        nc.scalar.mul(out=f_t[0:1, sl],
                                      in_=rows[32 * ch:32 * ch + 1, :],
                                      mul=-1.0 / C)
                        nc.scalar.mul(out=t0[:, sl],
                                      in_=rows[64 + 32 * ch:64 + 32 * ch + 1, :],
                                      mul=1.0 / C)
                    m2 = rowpool.tile([1, N], F32, tag="m2", name="m2")
                    nc.vector.tensor_tensor(out=m2, in0=mrow, in1=mrow, op=OP.mult)
                    varr = rowpool.tile([1, N], F32, tag="varr", name="varr")
                    nc.vector.tensor_tensor(out=varr, in0=t0, in1=m2, op=OP.subtract)
                    stdrow = rowpool.tile([1, N], F16, tag="stdrow", name="stdrow")
                    nc.scalar.activation(out=stdrow, in_=varr, func=AF.Sqrt,
                                         bias=eps_row, scale=1.0)
                    # DMA (partition-unconstrained) assembles row 1 of f_t
                    nc.sync.dma_start(out=f_t[1:2, :], in_=stdrow)
                    # per-token 1/std scale [128, NT]: DMA-spread the std
                    # row through DRAM, then one cheap DVE reciprocal
                    nc.sync.dma_start(out=ascr[:, :], in_=stdrow)
                    asp = rowpool.tile([128, NT], F16, tag="asp", name="asp")
                    nc.sync.dma_start(
                        out=asp,
                        in_=bass.AP(tensor=ascr[0:1, :].tensor, offset=0,
                                    ap=[[1, 128], [128, NT]]))
                    nc.vector.reciprocal(out=acol, in_=asp)

                with tc.tile_pool(name="ps_out", bufs=3, space="PSUM") as ps_out:
                    for n in range(NT):
                        po = ps_out.tile([128, C], F32, tag="po", name="po")
                        for c0, cw in ((0, 512), (512, C - 512)):
                            for k in range(KT):
                                nc.tensor.matmul(
                                    out=po[:, c0:c0 + cw],
                                    lhsT=att[k][:, n * 128:(n + 1) * 128],
                                    rhs=wo_t[k][:, c0:c0 + cw],
                                    start=(k == 0), stop=False,
                                )
                            nc.tensor.matmul(
                                out=po[:, c0:c0 + cw],
                                lhsT=f_t[:, n * 128:(n + 1) * 128],
                                rhs=extra_t[:, c0:c0 + cw],
                                start=False, stop=True,
                            )
                        yt = ypool.tile([128, C], F32, tag="yt", name="yt")
                        nc.vector.tensor_scalar(
                            out=yt, in0=po, scalar1=acol[:, n:n + 1], scalar2=None,
                            op0=OP.mult)
                        nc.sync.dma_start(out=y_d[n * 128:(n + 1) * 128, :], in_=yt)

    nc.compile()
    return nc


def prepare_in_maps(x, Wq, bq, Wk, bk, Wv, bv, Wo, bo, ln_g, ln_b):
    x = np.asarray(x, np.float32)
    Wq = np.asarray(Wq, np.float32); bq = np.asarray(bq, np.float32)
    Wk = np.asarray(Wk, np.float32)
    Wv = np.asarray(Wv, np.float32); bv = np.asarray(bv, np.float32)
    Wo = np.asarray(Wo, np.float32); bo = np.asarray(bo, np.float32)
    ln_g = np.asarray(ln_g, np.float32); ln_b = np.asarray(ln_b, np.float32)

    wq = np.ascontiguousarray(Wq.T) * SCALE
    wk = np.ascontiguousarray(Wk.T)
    wv = np.ascontiguousarray(Wv.T)            # [C, C]
    wv_ext = np.zeros((C, VW), np.float32)
    bve = np.zeros((VW,), np.float32)
    for h in range(H):
        wv_ext[:, h * 65: h * 65 + 64] = wv[:, h * 64:(h + 1) * 64]
        bve[h * 65: h * 65 + 64] = bv[h * 64:(h + 1) * 64]
        bve[h * 65 + 64] = 1.0                 # ones column for denominators
    # pair-blocked q/k weights: [wq_p | wk_p] per 128-channel head pair
    wqk = np.zeros((C, 2 * C), np.float32)
    for p in range(NP):
        wqk[:, p * 256: p * 256 + 128] = wq[:, p * 128:(p + 1) * 128]
        wqk[:, p * 256 + 128:(p + 1) * 256] = wk[:, p * 128:(p + 1) * 128]
    wo = ln_g[:, None] * np.ascontiguousarray(Wo.T)
    bo_eff = bo + ln_b @ Wo.T
    extra = np.stack([wo.sum(axis=0), bo_eff]).astype(np.float32)
    bqs = bq * SCALE

    f16 = np.float16
    shared = {"wqk": wqk.astype(f16), "wv": wv_ext.astype(f16),
              "wo": wo.astype(f16), "extra": extra.astype(f16),
              "bqs": bqs, "bve": bve.astype(f16),
              "onesd": np.ones(128, f16)}
    in_maps = []
    for b in range(B):
        xT = np.ascontiguousarray(x[b].T).astype(f16)   # [C, N]
        in_maps.append({"xt": xT, **shared})
    return in_maps


_NC_CACHE = []


def _get_nc():
    if not _NC_CACHE:
        _NC_CACHE.append(build_kernel())
    return _NC_CACHE[0]


def kernel(**inputs) -> np.ndarray:
    nc = _get_nc()
    in_maps = prepare_in_maps(**inputs)
    res = run_bass_kernel_spmd(nc, in_maps, core_ids=list(range(B)))
    return np.stack([res.results[b]["y"] for b in range(B)], axis=0)


# revision 11
# speedup vs baseline: 1.1840x; 1.1840x over previous
"""Trainium2 Bass kernel for nn_Attention_28269474742408.

Single-layer attention block: qkv projections -> softmax attention ->
layernorm -> output projection, for x [8, 1024, 768] (B=8, N=1024, C=768,
H=12 heads, D=64).

Strategy: data parallel over the batch - one batch element per NeuronCore
(8 cores). Everything on-chip per core; no collectives.

Per-core structure (all channel-major, "T" = [channel, token]):
  - Host pre-transposes x[b] -> xT [768, 1024] and all weights -> W.T so
    projections/attention never need on-device transposes.
  - v is projected token-major [1024, 780]: 12 heads x (64 v-cols + a ones
    column); the ones column makes the PV matmul emit softmax denominators
    for free.  The v projection is emitted in column thirds interleaved
    into the pair-0 attention iterations so it hides under the exp stream.
  - Attention runs an mt-outer loop per head PAIR: the two heads' score
    matmuls (K=64 contraction) are row-tiled into the top/bottom halves of
    the PE array (tile_position via base_partition 0/64) so they run
    concurrently - the pair's scores cost one matmul slot instead of two.
  - softmax skips max-subtraction (scores bounded ~|3|, exp can't
    overflow); exp output feeds flash-style PV accumulation in PSUM.
  - Per-head softmax denominators: reciprocal is computed on a DMA-spread
    [128, 8] layout (DVE reciprocal is ~8 cyc per free-dim element, so a
    [1, 1024] row costs 6.5us but the spread costs ~60ns), then broadcast
    back over 64 partitions via a DRAM bounce.  The PSUM accumulator is
    freed early by copying it to SBUF before the division.
  - q/k projections for pair p+1 are emitted as filler units inside pair
    p's iterations, keeping TensorE dense while ScalarE runs exp.
  - LayerNorm is folded into the output projection: gamma/beta folded into
    Wo/bo on the host; mean/var via ones-matmuls, 4-way column-tiled into
    one PSUM bank (partition offsets 0/32/64/96 run concurrently);
    the -mean*colsum(Wo) + sqrt(var+eps)*bo rank-2 correction rides the
    output matmul as an extra K=2 accumulation; the rsqrt scale is applied
    per-token on eviction (built by DMA-spreading the std row to [128, 8]
    and one DVE reciprocal).
  - A dummy exp at t=0 preloads the activation table under the input DMA.
"""
import numpy as np

import concourse.bacc as bacc
import concourse.bass as bass
import concourse.tile as tile
from concourse import bass_isa
from concourse import mybir
from concourse.bass_utils import run_bass_kernel_spmd

F32 = mybir.dt.float32
F16 = mybir.dt.float16
AF = mybir.ActivationFunctionType
OP = mybir.AluOpType

B, N, C, H, D = 8, 1024, 768, 12, 64
KT = C // 128          # 6 channel tiles
NT = N // 128          # 8 token tiles
NP = H // 2            # 6 head pairs
VW = H * (D + 1)       # 780: v plus per-head ones column
VT = VW // 3           # 260: v projection column third
SCALE = D ** -0.5
EPS = 1e-5


def build_kernel():
    nc = bacc.Bacc("TRN2", target_bir_lowering=False)

    xt_d = nc.dram_tensor("xt", (C, N), F16, kind="ExternalInput")
    wv_d = nc.dram_tensor("wv", (C, VW), F16, kind="ExternalInput")
    wqk_d = nc.dram_tensor("wqk", (C, 2 * C), F16, kind="ExternalInput")
    wo_d = nc.dram_tensor("wo", (C, C), F16, kind="ExternalInput")
    extra_d = nc.dram_tensor("extra", (2, C), F16, kind="ExternalInput")
    bqs_d = nc.dram_tensor("bqs", (C,), F32, kind="ExternalInput")
    bve_d = nc.dram_tensor("bve", (VW,), F16, kind="ExternalInput")
    ones_d = nc.dram_tensor("onesd", (128,), F16, kind="ExternalInput")

    y_d = nc.dram_tensor("y", (N, C), F32, kind="ExternalOutput")
    rscr = nc.dram_tensor("rscr", (H, N), F32)     # internal: Z bounce
    rsci = nc.dram_tensor("rsci", (H, N), F32)     # internal: 1/Z bounce

    with tile.TileContext(nc) as tc:
        with tc.tile_pool(name="persist", bufs=1) as pp, \
             tc.tile_pool(name="attp", bufs=1) as attp, \
             tc.tile_pool(name="wop", bufs=1) as wop, \
             tc.tile_pool(name="sqp", bufs=1) as sqp:

            # ---- constants (loads emitted after the xT DMA, below) ----
            extra_t = pp.tile([2, C], F16, tag="extra", name="extra")
            bqs_t = [pp.tile([128, 1], F32, tag=f"bqs{m}", name=f"bqs{m}")
                     for m in range(NP)]
            ones_col = pp.tile([128, 1], F16, tag="ones_col", name="ones_col")
            eps_row = pp.tile([1, 1], F32, tag="eps_row", name="eps_row")
            warm_row = pp.tile([1, 1], F32, tag="warm_row", name="warm_row")
            nc.vector.memset(eps_row, EPS)
            # dummy exp: forces the exp act-table load under the input DMA
            nc.scalar.activation(out=warm_row, in_=eps_row, func=AF.Exp)
            # bias for v broadcast across all partitions (applied on evict)
            bve_bc = pp.tile([128, VW], F16, tag="bve_bc", name="bve_bc")

            att = [attp.tile([128, N], F16, tag=f"att{k}", name=f"att{k}")
                   for k in range(KT)]
            f_t = pp.tile([2, N], F16, tag="f_t", name="f_t")
            acol = pp.tile([128, NT], F32, tag="acol", name="acol")

            with tc.tile_pool(name="xtp", bufs=1) as xtp, \
                 tc.tile_pool(name="vp", bufs=1) as vp, \
                 tc.tile_pool(name="wvp", bufs=1) as wvp, \
                 tc.tile_pool(name="pairw", bufs=3) as pairw, \
                 tc.tile_pool(name="qkpair", bufs=3) as qkpair, \
                 tc.tile_pool(name="epool", bufs=4) as epool, \
                 tc.tile_pool(name="cpool", bufs=2) as cpool, \
                 tc.tile_pool(name="zspp", bufs=2) as zspp, \
                 tc.tile_pool(name="rbcp", bufs=2) as rbcp:

                xt = [xtp.tile([128, N], F16, tag=f"xt{k}", name=f"xt{k}")
                      for k in range(KT)]
                for k in range(KT):
                    nc.sync.dma_start(out=xt[k], in_=xt_d[k * 128:(k + 1) * 128, :])
                vt = [vp.tile([128, VW], F16, tag=f"vt{n}", name=f"vt{n}")
                      for n in range(NT)]
                wv_t = [wvp.tile([128, VW], F16, tag=f"wv{k}", name=f"wv{k}")
                        for k in range(KT)]
                wo_t = [wop.tile([128, C], F16, tag=f"wo{k}", name=f"wo{k}")
                        for k in range(KT)]
                sq = [sqp.tile([128, N], F16, tag=f"sq{k}", name=f"sq{k}")
                      for k in range(KT)]

                with tc.tile_pool(name="ps_s", bufs=2, space="PSUM") as ps_s, \
                     tc.tile_pool(name="ps_att", bufs=2, space="PSUM") as ps_att:
                    qk_tiles = {}

                    def load_pair_w(p):
                        pw = [pairw.tile([128, 256], F16, tag=f"pw{k}",
                                         name=f"pw{k}") for k in range(KT)]
                        for k in range(KT):
                            nc.sync.dma_start(
                                out=pw[k],
                                in_=wqk_d[k * 128:(k + 1) * 128,
                                          p * 256:(p + 1) * 256])
                        qt = qkpair.tile([128, N], F16, tag="qtp", name="qtp")
                        kt = qkpair.tile([128, N], F16, tag="ktp", name="ktp")
                        qk_tiles[p] = (pw, qt, kt)

                    def emit_proj_group(p, which, ch):
                        # one q-or-k projection chunk for pair p: [128, 512]
                        pw, qt, kt = qk_tiles[p]
                        off = 0 if which == "q" else 128
                        pg = ps_s.tile([128, 512], F32, tag="sp", name="pg")
                        for k in range(KT):
                            nc.tensor.matmul(
                                out=pg, lhsT=pw[k][:, off:off + 128],
                                rhs=xt[k][:, ch * 512:(ch + 1) * 512],
                                start=(k == 0), stop=(k == KT - 1),
                            )
                        if which == "q":
                            nc.vector.tensor_scalar(
                                out=qt[:, ch * 512:(ch + 1) * 512], in0=pg,
                                scalar1=bqs_t[p], scalar2=None, op0=OP.add,
                            )
                        else:
                            nc.vector.tensor_copy(
                                out=kt[:, ch * 512:(ch + 1) * 512], in_=pg)

                    def emit_v_unit(n, c0):
                        # one v-projection third: vt[n][:, c0:c0+VT]
                        pv = ps_s.tile([128, VT], F32, tag="sp", name="pv")
                        for k in range(KT):
                            nc.tensor.matmul(
                                out=pv,
                                lhsT=xt[k][:, n * 128:(n + 1) * 128],
                                rhs=wv_t[k][:, c0:c0 + VT],
                                start=(k == 0), stop=(k == KT - 1),
                            )
                        nc.vector.tensor_tensor(
                            out=vt[n][:, c0:c0 + VT], in0=pv,
                            in1=bve_bc[:, c0:c0 + VT], op=OP.add)

                    # ---- lead-in: pair-0 q/k projections ----
                    load_pair_w(0)
                    # small consts after the latency-critical xT/pw0 loads
                    for m in range(NP):
                        nc.sync.dma_start(
                            out=bqs_t[m],
                            in_=bqs_d[m * 128:(m + 1) * 128].unsqueeze(1))
                    nc.sync.dma_start(out=ones_col, in_=ones_d[:].unsqueeze(1))
                    nc.sync.dma_start(
                        out=bve_bc,
                        in_=bass.AP(tensor=bve_d[:].tensor, offset=0,
                                    ap=[[0, 128], [1, VW]]))
                    for k in range(KT):
                        nc.sync.dma_start(out=wv_t[k],
                                          in_=wv_d[k * 128:(k + 1) * 128, :])
                    for which, ch in (("q", 0), ("q", 1), ("k", 0), ("k", 1)):
                        emit_proj_group(0, which, ch)

                    # ---- filler queue: one unit per (pair, mt) iteration.
                    # v thirds: A (cols 0-259, pairs 0-1) during pair 0;
                    # B (260-519, pairs 2-3) during pair 1; C (520-779,
                    # pairs 4-5) during pair 2.  q/k projections for pair
                    # p+1 ride the first four iterations of pair p.
                    fillers = {}
                    for mt in range(NT):
                        fillers[(0, mt)] = [lambda n=mt: emit_v_unit(n, 0)]
                        fillers[(1, mt)] = [lambda n=mt: emit_v_unit(n, VT)]
                        fillers[(2, mt)] = [lambda n=mt: emit_v_unit(n, 2 * VT)]
                        fillers[(3, mt)] = []
                        fillers[(4, mt)] = []
                        fillers[(5, mt)] = []
                    for p in range(NP - 1):
                        for i, (which, ch) in enumerate(
                                (("q", 0), ("q", 1), ("k", 0), ("k", 1))):
                            fillers[(p, i)].append(
                                lambda p=p, w=which, c=ch:
                                    emit_proj_group(p + 1, w, c))

                    # ---- attention: flat software-pipelined loop ----
                    # Per iteration g=(p,mt): fillers, QK, exp, then the PV
                    # of the PREVIOUS iteration (so PV never sits between
                    # exp(h1,i) and the next QK on the critical path).
                    # Each pair's denominator chain (copy -> spread ->
                    # reciprocal -> broadcast) is emitted at the next pair
                    # boundary, and its divides one pair later still, so
                    # the ~6us DMA round-trip never blocks the DVE FIFO.
                    pa_map, ev_map = {}, {}

                    def evict_block(p):
                        ev = []
                        for hh in range(2):
                            h = 2 * p + hh
                            cp = cpool.tile([65, N], F32, tag="cp", name="cp")
                            nc.vector.tensor_copy(out=cp, in_=pa_map[p][hh])
                            nc.sync.dma_start(out=rscr[h:h + 1, :],
                                              in_=cp[64:65, :])
                            zsp = zspp.tile([128, NT], F32, tag="zsp",
                                            name="zsp")
                            src = rscr[h:h + 1, :]
                            nc.sync.dma_start(
                                out=zsp,
                                in_=bass.AP(tensor=src.tensor, offset=src.offset,
                                            ap=[[1, 128], [128, NT]]))
                            zspi = zspp.tile([128, NT], F32, tag="zspi",
                                             name="zspi")
                            nc.vector.reciprocal(out=zspi, in_=zsp)
                            dst = rsci[h:h + 1, :]
                            nc.sync.dma_start(
                                out=bass.AP(tensor=dst.tensor, offset=dst.offset,
                                            ap=[[1, 128], [128, NT]]),
                                in_=zspi)
                            rbc = rbcp.tile([64, N], F32, tag="rbc", name="rbc")
                            nc.sync.dma_start(
                                out=rbc,
                                in_=bass.AP(tensor=dst.tensor, offset=dst.offset,
                                            ap=[[0, 64], [1, N]]))
                            ev.append((cp, rbc))
                        ev_map[p] = ev

                    def div_block(p):
                        for hh in range(2):
                            cp, rbc = ev_map[p][hh]
                            nc.vector.tensor_tensor(
                                out=att[p][hh * 64:hh * 64 + 64, :],
                                in0=cp[0:64, :], in1=rbc, op=OP.mult)
                        del ev_map[p]
                        nc.vector.tensor_tensor(out=sq[p], in0=att[p],
                                                in1=att[p], op=OP.mult)

                    def emit_pv(p, mt, e_t):
                        if mt == 0:
                            pa_map[p] = [ps_att.tile([65, N], F32, tag="pa",
                                                     name=f"pa{hh}")
                                         for hh in range(2)]
                        for hh in range(2):
                            h = 2 * p + hh
                            for ch in range(2):
                                nc.tensor.matmul(
                                    out=pa_map[p][hh][:, ch * 512:(ch + 1) * 512],
                                    lhsT=vt[mt][:, h * 65:(h + 1) * 65],
                                    rhs=e_t[hh][:, ch * 512:(ch + 1) * 512],
                                    start=(mt == 0), stop=(mt == NT - 1),
                                )

                    prev = None
                    for g in range(NP * NT):
                        p, mt = divmod(g, NT)
                        if mt == 0:
                            if p + 1 < NP:
                                load_pair_w(p + 1)
                            if p == 2:
                                # wo needed only by the output projection
                                for k in range(KT):
                                    nc.sync.dma_start(
                                        out=wo_t[k],
                                        in_=wo_d[k * 128:(k + 1) * 128, :])
                                nc.sync.dma_start(out=extra_t, in_=extra_d[:, :])
                        _, qt, kt = qk_tiles[p]
                        for fn in fillers.pop((p, mt), ()):
                            fn()
                        s_t = [ps_s.tile([128, N], F32, tag="sp",
                                         name=f"s{hh}") for hh in range(2)]
                        # row-tiled pair scores: head hh uses PE array
                        # rows 64*hh..64*hh+63 - both run concurrently
                        for ch in range(2):
                            for hh in range(2):
                                hr = hh * 64
                                nc.tensor.matmul(
                                    out=s_t[hh][:, ch * 512:(ch + 1) * 512],
                                    lhsT=kt[hr:hr + 64, mt * 128:(mt + 1) * 128],
                                    rhs=qt[hr:hr + 64, ch * 512:(ch + 1) * 512],
                                    start=True, stop=True,
                                )
                        e_t = []
                        for hh in range(2):
                            e = epool.tile([128, N], F16, tag="e", name="e")
                            nc.scalar.activation(out=e, in_=s_t[hh], func=AF.Exp)
                            e_t.append(e)
                        if prev is not None:
                            pp, pmt, pe = prev
                            emit_pv(pp, pmt, pe)
                            if pmt == NT - 1:
                                evict_block(pp)
                                if pp >= 1:
                                    div_block(pp - 1)
                        prev = (p, mt, e_t)
                    pp, pmt, pe = prev
                    emit_pv(pp, pmt, pe)
                    evict_block(pp)
                    div_block(pp - 1)
                    div_block(pp)

            # ---- phase C: LN stats + output projection ----
            with tc.tile_pool(name="rowpool", bufs=1) as rowpool, \
                 tc.tile_pool(name="ypool", bufs=3) as ypool:

                with tc.tile_pool(name="ps_row", bufs=1, space="PSUM") as ps_row:
                    # 4-way column-tiled stats in one PSUM bank: rows
                    # 0/32/64/96 hold sum(att) ch0/ch1, sum(att^2) ch0/ch1
                    rows = ps_row.tile([97, 512], F32, tag="rows", name="rows")
                    for k in range(KT):
                        for j, (srcs, ch) in enumerate(
                                ((att, 0), (att, 1), (sq, 0), (sq, 1))):
                            nc.tensor.matmul(
                                out=rows[32 * j:32 * j + 1, :],
                                lhsT=ones_col,
                                rhs=srcs[k][:, ch * 512:(ch + 1) * 512],
                                start=(k == 0), stop=(k == KT - 1),
                                tile_position=(0, 32 * j),
                            )
                    mrow = rowpool.tile([1, N], F32, tag="mrow", name="mrow")
                    t0 = rowpool.tile([1, N], F32, tag="t0", name="t0")
                    for ch in range(2):
                        sl = slice(ch * 512, (ch + 1) * 512)
                        nc.scalar.mul(out=mrow[:, sl],
                                      in_=rows[32 * ch:32 * ch + 1, :],
                                      mul=1.0 / C)
                        nc.scalar.mul(out=f_t[0:1, sl],
                                      in_=rows[32 * ch:32 * ch + 1, :],
                                      mul=-1.0 / C)
                        nc.scalar.mul(out=t0[:, sl],
                                      in_=rows[64 + 32 * ch:64 + 32 * ch + 1, :],
                                      mul=1.0 / C)
                    m2 = rowpool.tile([1, N], F32, tag="m2", name="m2")
                    nc.vector.tensor_tensor(out=m2, in0=mrow, in1=mrow, op=OP.mult)
                    varr = rowpool.tile([1, N], F32, tag="varr", name="varr")
                    nc.vector.tensor_tensor(out=varr, in0=t0, in1=m2, op=OP.subtract)
                    stdrow = rowpool.tile([1, N], F16, tag="stdrow", name="stdrow")
                    nc.scalar.activation(out=stdrow, in_=varr, func=AF.Sqrt,
                                         bias=eps_row, scale=1.0)
                    # DMA (partition-unconstrained) assembles row 1 of f_t
                    nc.sync.dma_start(out=f_t[1:2, :], in_=stdrow)
                    # per-token 1/std scale [128, NT]: transpose the std row
                    # via NT tiny K=1 matmuls, then one PSUM reciprocal -
                    # zero DMA hops on the output-projection critical path
                    with tc.tile_pool(name="ps_pt", bufs=1,
                                      space="PSUM") as ps_pt:
                        pt = ps_pt.tile([128, NT], F32, tag="pt", name="pt")
                        for j in range(NT):
                            nc.tensor.matmul(
                                out=pt[:, j:j + 1],
                                lhsT=stdrow[0:1, j * 128:(j + 1) * 128],
                                rhs=ones_col[0:1, 0:1],
                                start=True, stop=True)
                        nc.vector.reciprocal(out=acol, in_=pt)

                with tc.tile_pool(name="ps_out", bufs=3, space="PSUM") as ps_out:
                    for n in range(NT):
                        po = ps_out.tile([128, C], F32, tag="po", name="po")
                        for c0, cw in ((0, 512), (512, C - 512)):
                            for k in range(KT):
                                nc.tensor.matmul(
                                    out=po[:, c0:c0 + cw],
                                    lhsT=att[k][:, n * 128:(n + 1) * 128],
                                    rhs=wo_t[k][:, c0:c0 + cw],
                                    start=(k == 0), stop=False,
                                )
                            nc.tensor.matmul(
                                out=po[:, c0:c0 + cw],
                                lhsT=f_t[:, n * 128:(n + 1) * 128],
                                rhs=extra_t[:, c0:c0 + cw],
                                start=False, stop=True,
                            )
                        yt = ypool.tile([128, C], F32, tag="yt", name="yt")
                        nc.vector.tensor_scalar(
                            out=yt, in0=po, scalar1=acol[:, n:n + 1], scalar2=None,
                            op0=OP.mult)
                        nc.sync.dma_start(out=y_d[n * 128:(n + 1) * 128, :], in_=yt)

    nc.compile()
    return nc


def prepare_in_maps(x, Wq, bq, Wk, bk, Wv, bv, Wo, bo, ln_g, ln_b):
    x = np.asarray(x, np.float32)
    Wq = np.asarray(Wq, np.float32); bq = np.asarray(bq, np.float32)
    Wk = np.asarray(Wk, np.float32)
    Wv = np.asarray(Wv, np.float32); bv = np.asarray(bv, np.float32)
    Wo = np.asarray(Wo, np.float32); bo = np.asarray(bo, np.float32)
    ln_g = np.asarray(ln_g, np.float32); ln_b = np.asarray(ln_b, np.float32)

    wq = np.ascontiguousarray(Wq.T) * SCALE
    wk = np.ascontiguousarray(Wk.T)
    wv = np.ascontiguousarray(Wv.T)            # [C, C]
    wv_ext = np.zeros((C, VW), np.float32)
    bve = np.zeros((VW,), np.float32)
    for h in range(H):
        wv_ext[:, h * 65: h * 65 + 64] = wv[:, h * 64:(h + 1) * 64]
        bve[h * 65: h * 65 + 64] = bv[h * 64:(h + 1) * 64]
        bve[h * 65 + 64] = 1.0                 # ones column for denominators
    # pair-blocked q/k weights: [wq_p | wk_p] per 128-channel head pair
    wqk = np.zeros((C, 2 * C), np.float32)
    for p in range(NP):
        wqk[:, p * 256: p * 256 + 128] = wq[:, p * 128:(p + 1) * 128]
        wqk[:, p * 256 + 128:(p + 1) * 256] = wk[:, p * 128:(p + 1) * 128]
    wo = ln_g[:, None] * np.ascontiguousarray(Wo.T)
    bo_eff = bo + ln_b @ Wo.T
    extra = np.stack([wo.sum(axis=0), bo_eff]).astype(np.float32)
    bqs = bq * SCALE

    f16 = np.float16
    shared = {"wqk": wqk.astype(f16), "wv": wv_ext.astype(f16),
              "wo": wo.astype(f16), "extra": extra.astype(f16),
              "bqs": bqs, "bve": bve.astype(f16),
              "onesd": np.ones(128, f16)}
    in_maps = []
    for b in range(B):
        xT = np.ascontiguousarray(x[b].T).astype(f16)   # [C, N]
        in_maps.append({"xt": xT, **shared})
    return in_maps


_NC_CACHE = []


def _get_nc():
    if not _NC_CACHE:
        _NC_CACHE.append(build_kernel())
    return _NC_CACHE[0]


def kernel(**inputs) -> np.ndarray:
    nc = _get_nc()
    in_maps = prepare_in_maps(**inputs)
    res = run_bass_kernel_spmd(nc, in_maps, core_ids=list(range(B)))
    return np.stack([res.results[b]["y"] for b in range(B)], axis=0)


# revision 12
# speedup vs baseline: 1.1857x; 1.0015x over previous
"""Trainium2 Bass kernel for nn_Attention_28269474742408.

Single-layer attention block: qkv projections -> softmax attention ->
layernorm -> output projection, for x [8, 1024, 768] (B=8, N=1024, C=768,
H=12 heads, D=64).

Strategy: data parallel over the batch - one batch element per NeuronCore
(8 cores). Everything on-chip per core; no collectives.

Per-core structure (all channel-major, "T" = [channel, token]):
  - Host pre-transposes x[b] -> xT [768, 1024] and all weights -> W.T so
    projections/attention never need on-device transposes.
  - v is projected token-major [1024, 780]: 12 heads x (64 v-cols + a ones
    column); the ones column makes the PV matmul emit softmax denominators
    for free.  The v projection is emitted in column thirds interleaved
    into the pair-0 attention iterations so it hides under the exp stream.
  - Attention runs an mt-outer loop per head PAIR: the two heads' score
    matmuls (K=64 contraction) are row-tiled into the top/bottom halves of
    the PE array (tile_position via base_partition 0/64) so they run
    concurrently - the pair's scores cost one matmul slot instead of two.
  - softmax skips max-subtraction (scores bounded ~|3|, exp can't
    overflow); exp output feeds flash-style PV accumulation in PSUM.
  - Per-head softmax denominators: reciprocal is computed on a DMA-spread
    [128, 8] layout (DVE reciprocal is ~8 cyc per free-dim element, so a
    [1, 1024] row costs 6.5us but the spread costs ~60ns), then broadcast
    back over 64 partitions via a DRAM bounce.  The PSUM accumulator is
    freed early by copying it to SBUF before the division.
  - q/k projections for pair p+1 are emitted as filler units inside pair
    p's iterations, keeping TensorE dense while ScalarE runs exp.
  - LayerNorm is folded into the output projection: gamma/beta folded into
    Wo/bo on the host; mean/var via ones-matmuls, 4-way column-tiled into
    one PSUM bank (partition offsets 0/32/64/96 run concurrently);
    the -mean*colsum(Wo) + sqrt(var+eps)*bo rank-2 correction rides the
    output matmul as an extra K=2 accumulation; the rsqrt scale is applied
    per-token on eviction (built by DMA-spreading the std row to [128, 8]
    and one DVE reciprocal).
  - A dummy exp at t=0 preloads the activation table under the input DMA.
"""
import numpy as np

import concourse.bacc as bacc
import concourse.bass as bass
import concourse.tile as tile
from concourse import bass_isa
from concourse import mybir
from concourse.bass_utils import run_bass_kernel_spmd

F32 = mybir.dt.float32
F16 = mybir.dt.float16
AF = mybir.ActivationFunctionType
OP = mybir.AluOpType

B, N, C, H, D = 8, 1024, 768, 12, 64
KT = C // 128          # 6 channel tiles
NT = N // 128          # 8 token tiles
NP = H // 2            # 6 head pairs
VW = H * (D + 1)       # 780: v plus per-head ones column
VT = VW // 3           # 260: v projection column third
SCALE = D ** -0.5
EPS = 1e-5


def build_kernel():
    nc = bacc.Bacc("TRN2", target_bir_lowering=False)

    xt_d = nc.dram_tensor("xt", (C, N), F16, kind="ExternalInput")
    wv_d = nc.dram_tensor("wv", (C, VW), F16, kind="ExternalInput")
    wqk_d = nc.dram_tensor("wqk", (C, 2 * C), F16, kind="ExternalInput")
    wo_d = nc.dram_tensor("wo", (C, C), F16, kind="ExternalInput")
    extra_d = nc.dram_tensor("extra", (2, C), F16, kind="ExternalInput")
    bqs_d = nc.dram_tensor("bqs", (C,), F32, kind="ExternalInput")
    bve_d = nc.dram_tensor("bve", (VW,), F16, kind="ExternalInput")
    ones_d = nc.dram_tensor("onesd", (128,), F16, kind="ExternalInput")

    y_d = nc.dram_tensor("y", (N, C), F32, kind="ExternalOutput")
    rscr = nc.dram_tensor("rscr", (H, N), F32)     # internal: Z bounce
    rsci = nc.dram_tensor("rsci", (H, N), F32)     # internal: 1/Z bounce

    with tile.TileContext(nc) as tc:
        with tc.tile_pool(name="persist", bufs=1) as pp, \
             tc.tile_pool(name="attp", bufs=1) as attp, \
             tc.tile_pool(name="wop", bufs=1) as wop, \
             tc.tile_pool(name="sqp", bufs=1) as sqp:

            # ---- constants (loads emitted after the xT DMA, below) ----
            extra_t = pp.tile([2, C], F16, tag="extra", name="extra")
            bqs_t = [pp.tile([128, 1], F32, tag=f"bqs{m}", name=f"bqs{m}")
                     for m in range(NP)]
            ones_col = pp.tile([128, 1], F16, tag="ones_col", name="ones_col")
            eps_row = pp.tile([1, 1], F32, tag="eps_row", name="eps_row")
            warm_row = pp.tile([1, 1], F32, tag="warm_row", name="warm_row")
            nc.vector.memset(eps_row, EPS)
            # dummy exp: forces the exp act-table load under the input DMA
            nc.scalar.activation(out=warm_row, in_=eps_row, func=AF.Exp)
            # bias for v broadcast across all partitions (applied on evict)
            bve_bc = pp.tile([128, VW], F16, tag="bve_bc", name="bve_bc")

            att = [attp.tile([128, N], F16, tag=f"att{k}", name=f"att{k}")
                   for k in range(KT)]
            f_t = pp.tile([2, N], F16, tag="f_t", name="f_t")
            acol = pp.tile([128, NT], F32, tag="acol", name="acol")

            with tc.tile_pool(name="xtp", bufs=1) as xtp, \
                 tc.tile_pool(name="vp", bufs=1) as vp, \
                 tc.tile_pool(name="wvp", bufs=1) as wvp, \
                 tc.tile_pool(name="pairw", bufs=3) as pairw, \
                 tc.tile_pool(name="qkpair", bufs=3) as qkpair, \
                 tc.tile_pool(name="epool", bufs=4) as epool, \
                 tc.tile_pool(name="cpool", bufs=2) as cpool, \
                 tc.tile_pool(name="zspp", bufs=2) as zspp, \
                 tc.tile_pool(name="rbcp", bufs=2) as rbcp:

                xt = [xtp.tile([128, N], F16, tag=f"xt{k}", name=f"xt{k}")
                      for k in range(KT)]
                for k in range(KT):
                    nc.sync.dma_start(out=xt[k], in_=xt_d[k * 128:(k + 1) * 128, :])
                vt = [vp.tile([128, VW], F16, tag=f"vt{n}", name=f"vt{n}")
                      for n in range(NT)]
                wv_t = [wvp.tile([128, VW], F16, tag=f"wv{k}", name=f"wv{k}")
                        for k in range(KT)]
                wo_t = [wop.tile([128, C], F16, tag=f"wo{k}", name=f"wo{k}")
                        for k in range(KT)]
                sq = [sqp.tile([128, N], F16, tag=f"sq{k}", name=f"sq{k}")
                      for k in range(KT)]

                with tc.tile_pool(name="ps_s", bufs=2, space="PSUM") as ps_s, \
                     tc.tile_pool(name="ps_att", bufs=2, space="PSUM") as ps_att:
                    qk_tiles = {}

                    def load_pair_w(p):
                        pw = [pairw.tile([128, 256], F16, tag=f"pw{k}",
                                         name=f"pw{k}") for k in range(KT)]
                        for k in range(KT):
                            nc.sync.dma_start(
                                out=pw[k],
                                in_=wqk_d[k * 128:(k + 1) * 128,
                                          p * 256:(p + 1) * 256])
                        qt = qkpair.tile([128, N], F16, tag="qtp", name="qtp")
                        kt = qkpair.tile([128, N], F16, tag="ktp", name="ktp")
                        qk_tiles[p] = (pw, qt, kt)

                    def emit_proj_group(p, which, ch):
                        # one q-or-k projection chunk for pair p: [128, 512]
                        pw, qt, kt = qk_tiles[p]
                        off = 0 if which == "q" else 128
                        pg = ps_s.tile([128, 512], F32, tag="sp", name="pg")
                        for k in range(KT):
                            nc.tensor.matmul(
                                out=pg, lhsT=pw[k][:, off:off + 128],
                                rhs=xt[k][:, ch * 512:(ch + 1) * 512],
                                start=(k == 0), stop=(k == KT - 1),
                            )
                        if which == "q":
                            nc.vector.tensor_scalar(
                                out=qt[:, ch * 512:(ch + 1) * 512], in0=pg,
                                scalar1=bqs_t[p], scalar2=None, op0=OP.add,
                            )
                        else:
                            nc.vector.tensor_copy(
                                out=kt[:, ch * 512:(ch + 1) * 512], in_=pg)

                    def emit_v_unit(n, c0):
                        # one v-projection third: vt[n][:, c0:c0+VT]
                        pv = ps_s.tile([128, VT], F32, tag="sp", name="pv")
                        for k in range(KT):
                            nc.tensor.matmul(
                                out=pv,
                                lhsT=xt[k][:, n * 128:(n + 1) * 128],
                                rhs=wv_t[k][:, c0:c0 + VT],
                                start=(k == 0), stop=(k == KT - 1),
                            )
                        nc.vector.tensor_tensor(
                            out=vt[n][:, c0:c0 + VT], in0=pv,
                            in1=bve_bc[:, c0:c0 + VT], op=OP.add)

                    # ---- lead-in: pair-0 q/k projections ----
                    load_pair_w(0)
                    # small consts after the latency-critical xT/pw0 loads
                    for m in range(NP):
                        nc.sync.dma_start(
                            out=bqs_t[m],
                            in_=bqs_d[m * 128:(m + 1) * 128].unsqueeze(1))
                    nc.sync.dma_start(out=ones_col, in_=ones_d[:].unsqueeze(1))
                    nc.sync.dma_start(
                        out=bve_bc,
                        in_=bass.AP(tensor=bve_d[:].tensor, offset=0,
                                    ap=[[0, 128], [1, VW]]))
                    for k in range(KT):
                        nc.sync.dma_start(out=wv_t[k],
                                          in_=wv_d[k * 128:(k + 1) * 128, :])
                    for which, ch in (("q", 0), ("q", 1), ("k", 0), ("k", 1)):
                        emit_proj_group(0, which, ch)

                    # ---- filler queue: one unit per (pair, mt) iteration.
                    # v thirds: A (cols 0-259, pairs 0-1) during pair 0;
                    # B (260-519, pairs 2-3) during pair 1; C (520-779,
                    # pairs 4-5) during pair 2.  q/k projections for pair
                    # p+1 ride the first four iterations of pair p.
                    fillers = {}
                    for mt in range(NT):
                        fillers[(0, mt)] = [lambda n=mt: emit_v_unit(n, 0)]
                        fillers[(1, mt)] = [lambda n=mt: emit_v_unit(n, VT)]
                        fillers[(2, mt)] = [lambda n=mt: emit_v_unit(n, 2 * VT)]
                        fillers[(3, mt)] = []
                        fillers[(4, mt)] = []
                        fillers[(5, mt)] = []
                    for p in range(NP - 1):
                        for i, (which, ch) in enumerate(
                                (("q", 0), ("q", 1), ("k", 0), ("k", 1))):
                            fillers[(p, i)].append(
                                lambda p=p, w=which, c=ch:
                                    emit_proj_group(p + 1, w, c))

                    # ---- attention: flat software-pipelined loop ----
                    # Per iteration g=(p,mt): fillers, QK, exp, then the PV
                    # of the PREVIOUS iteration (so PV never sits between
                    # exp(h1,i) and the next QK on the critical path).
                    # Each pair's denominator chain (copy -> spread ->
                    # reciprocal -> broadcast) is emitted at the next pair
                    # boundary, and its divides one pair later still, so
                    # the ~6us DMA round-trip never blocks the DVE FIFO.
                    pa_map, ev_map = {}, {}

                    def evict_block(p):
                        ev = []
                        for hh in range(2):
                            h = 2 * p + hh
                            cp = cpool.tile([65, N], F32, tag="cp", name="cp")
                            nc.vector.tensor_copy(out=cp, in_=pa_map[p][hh])
                            nc.gpsimd.dma_start(out=rscr[h:h + 1, :],
                                              in_=cp[64:65, :])
                            zsp = zspp.tile([128, NT], F32, tag="zsp",
                                            name="zsp")
                            src = rscr[h:h + 1, :]
                            nc.gpsimd.dma_start(
                                out=zsp,
                                in_=bass.AP(tensor=src.tensor, offset=src.offset,
                                            ap=[[1, 128], [128, NT]]))
                            zspi = zspp.tile([128, NT], F32, tag="zspi",
                                             name="zspi")
                            nc.vector.reciprocal(out=zspi, in_=zsp)
                            dst = rsci[h:h + 1, :]
                            nc.gpsimd.dma_start(
                                out=bass.AP(tensor=dst.tensor, offset=dst.offset,
                                            ap=[[1, 128], [128, NT]]),
                                in_=zspi)
                            rbc = rbcp.tile([64, N], F32, tag="rbc", name="rbc")
                            nc.gpsimd.dma_start(
                                out=rbc,
                                in_=bass.AP(tensor=dst.tensor, offset=dst.offset,
                                            ap=[[0, 64], [1, N]]))
                            ev.append((cp, rbc))
                        ev_map[p] = ev

                    def div_block(p):
                        for hh in range(2):
                            cp, rbc = ev_map[p][hh]
                            nc.vector.tensor_tensor(
                                out=att[p][hh * 64:hh * 64 + 64, :],
                                in0=cp[0:64, :], in1=rbc, op=OP.mult)
                        del ev_map[p]
                        nc.vector.tensor_tensor(out=sq[p], in0=att[p],
                                                in1=att[p], op=OP.mult)

                    def emit_pv(p, mt, e_t):
                        if mt == 0:
                            pa_map[p] = [ps_att.tile([65, N], F32, tag="pa",
                                                     name=f"pa{hh}")
                                         for hh in range(2)]
                        for hh in range(2):
                            h = 2 * p + hh
                            for ch in range(2):
                                nc.tensor.matmul(
                                    out=pa_map[p][hh][:, ch * 512:(ch + 1) * 512],
                                    lhsT=vt[mt][:, h * 65:(h + 1) * 65],
                                    rhs=e_t[hh][:, ch * 512:(ch + 1) * 512],
                                    start=(mt == 0), stop=(mt == NT - 1),
                                )

                    prev = None
                    for g in range(NP * NT):
                        p, mt = divmod(g, NT)
                        if mt == 0:
                            if p + 1 < NP:
                                load_pair_w(p + 1)
                            if p == 2:
                                # wo needed only by the output projection
                                for k in range(KT):
                                    nc.sync.dma_start(
                                        out=wo_t[k],
                                        in_=wo_d[k * 128:(k + 1) * 128, :])
                                nc.sync.dma_start(out=extra_t, in_=extra_d[:, :])
                        _, qt, kt = qk_tiles[p]
                        for fn in fillers.pop((p, mt), ()):
                            fn()
                        s_t = [ps_s.tile([128, N], F32, tag="sp",
                                         name=f"s{hh}") for hh in range(2)]
                        # row-tiled pair scores: head hh uses PE array
                        # rows 64*hh..64*hh+63 - both run concurrently
                        for ch in range(2):
                            for hh in range(2):
                                hr = hh * 64
                                nc.tensor.matmul(
                                    out=s_t[hh][:, ch * 512:(ch + 1) * 512],
                                    lhsT=kt[hr:hr + 64, mt * 128:(mt + 1) * 128],
                                    rhs=qt[hr:hr + 64, ch * 512:(ch + 1) * 512],
                                    start=True, stop=True,
                                )
                        e_t = []
                        for hh in range(2):
                            e = epool.tile([128, N], F16, tag="e", name="e")
                            nc.scalar.activation(out=e, in_=s_t[hh], func=AF.Exp)
                            e_t.append(e)
                        if prev is not None:
                            pp, pmt, pe = prev
                            emit_pv(pp, pmt, pe)
                            if pmt == NT - 1:
                                evict_block(pp)
                                if pp >= 1:
                                    div_block(pp - 1)
                        prev = (p, mt, e_t)
                    pp, pmt, pe = prev
                    emit_pv(pp, pmt, pe)
                    evict_block(pp)
                    div_block(pp - 1)
                    div_block(pp)

            # ---- phase C: LN stats + output projection ----
            with tc.tile_pool(name="rowpool", bufs=1) as rowpool, \
                 tc.tile_pool(name="ypool", bufs=3) as ypool:

                with tc.tile_pool(name="ps_row", bufs=1, space="PSUM") as ps_row:
                    # 4-way column-tiled stats in one PSUM bank: rows
                    # 0/32/64/96 hold sum(att) ch0/ch1, sum(att^2) ch0/ch1
                    rows = ps_row.tile([97, 512], F32, tag="rows", name="rows")
                    for k in range(KT):
                        for j, (srcs, ch) in enumerate(
                                ((att, 0), (att, 1), (sq, 0), (sq, 1))):
                            nc.tensor.matmul(
                                out=rows[32 * j:32 * j + 1, :],
                                lhsT=ones_col,
                                rhs=srcs[k][:, ch * 512:(ch + 1) * 512],
                                start=(k == 0), stop=(k == KT - 1),
                                tile_position=(0, 32 * j),
                            )
                    mrow = rowpool.tile([1, N], F32, tag="mrow", name="mrow")
                    t0 = rowpool.tile([1, N], F32, tag="t0", name="t0")
                    for ch in range(2):
                        sl = slice(ch * 512, (ch + 1) * 512)
                        nc.scalar.mul(out=mrow[:, sl],
                                      in_=rows[32 * ch:32 * ch + 1, :],
                                      mul=1.0 / C)
                        nc.scalar.mul(out=f_t[0:1, sl],
                                      in_=rows[32 * ch:32 * ch + 1, :],
                                      mul=-1.0 / C)
                        nc.scalar.mul(out=t0[:, sl],
                                      in_=rows[64 + 32 * ch:64 + 32 * ch + 1, :],
                                      mul=1.0 / C)
                    m2 = rowpool.tile([1, N], F32, tag="m2", name="m2")
                    nc.vector.tensor_tensor(out=m2, in0=mrow, in1=mrow, op=OP.mult)
                    varr = rowpool.tile([1, N], F32, tag="varr", name="varr")
                    nc.vector.tensor_tensor(out=varr, in0=t0, in1=m2, op=OP.subtract)
                    stdrow = rowpool.tile([1, N], F16, tag="stdrow", name="stdrow")
                    nc.scalar.activation(out=stdrow, in_=varr, func=AF.Sqrt,
                                         bias=eps_row, scale=1.0)
                    # DMA (partition-unconstrained) assembles row 1 of f_t
                    nc.sync.dma_start(out=f_t[1:2, :], in_=stdrow)
                    # per-token 1/std scale [128, NT]: transpose the std row
                    # via NT tiny K=1 matmuls, then one PSUM reciprocal -
                    # zero DMA hops on the output-projection critical path
                    with tc.tile_pool(name="ps_pt", bufs=1,
                                      space="PSUM") as ps_pt:
                        pt = ps_pt.tile([128, NT], F32, tag="pt", name="pt")
                        for j in range(NT):
                            nc.tensor.matmul(
                                out=pt[:, j:j + 1],
                                lhsT=stdrow[0:1, j * 128:(j + 1) * 128],
                                rhs=ones_col[0:1, 0:1],
                                start=True, stop=True)
                        nc.vector.reciprocal(out=acol, in_=pt)

                with tc.tile_pool(name="ps_out", bufs=3, space="PSUM") as ps_out:
                    for n in range(NT):
                        po = ps_out.tile([128, C], F32, tag="po", name="po")
                        for c0, cw in ((0, 512), (512, C - 512)):
                            for k in range(KT):
                                nc.tensor.matmul(
                                    out=po[:, c0:c0 + cw],
                                    lhsT=att[k][:, n * 128:(n + 1) * 128],
                                    rhs=wo_t[k][:, c0:c0 + cw],
                                    start=(k == 0), stop=False,
                                )
                            nc.tensor.matmul(
                                out=po[:, c0:c0 + cw],
                                lhsT=f_t[:, n * 128:(n + 1) * 128],
                                rhs=extra_t[:, c0:c0 + cw],
                                start=False, stop=True,
                            )
                        yt = ypool.tile([128, C], F32, tag="yt", name="yt")
                        nc.vector.tensor_scalar(
                            out=yt, in0=po, scalar1=acol[:, n:n + 1], scalar2=None,
                            op0=OP.mult)
                        nc.sync.dma_start(out=y_d[n * 128:(n + 1) * 128, :], in_=yt)

    nc.compile()
    return nc


def prepare_in_maps(x, Wq, bq, Wk, bk, Wv, bv, Wo, bo, ln_g, ln_b):
    x = np.asarray(x, np.float32)
    Wq = np.asarray(Wq, np.float32); bq = np.asarray(bq, np.float32)
    Wk = np.asarray(Wk, np.float32)
    Wv = np.asarray(Wv, np.float32); bv = np.asarray(bv, np.float32)
    Wo = np.asarray(Wo, np.float32); bo = np.asarray(bo, np.float32)
    ln_g = np.asarray(ln_g, np.float32); ln_b = np.asarray(ln_b, np.float32)

    wq = np.ascontiguousarray(Wq.T) * SCALE
    wk = np.ascontiguousarray(Wk.T)
    wv = np.ascontiguousarray(Wv.T)            # [C, C]
    wv_ext = np.zeros((C, VW), np.float32)
    bve = np.zeros((VW,), np.float32)
    for h in range(H):
        wv_ext[:, h * 65: h * 65 + 64] = wv[:, h * 64:(h + 1) * 64]
        bve[h * 65: h * 65 + 64] = bv[h * 64:(h + 1) * 64]
        bve[h * 65 + 64] = 1.0                 # ones column for denominators
    # pair-blocked q/k weights: [wq_p | wk_p] per 128-channel head pair
    wqk = np.zeros((C, 2 * C), np.float32)
    for p in range(NP):
        wqk[:, p * 256: p * 256 + 128] = wq[:, p * 128:(p + 1) * 128]
        wqk[:, p * 256 + 128:(p + 1) * 256] = wk[:, p * 128:(p + 1) * 128]
    wo = ln_g[:, None] * np.ascontiguousarray(Wo.T)
    bo_eff = bo + ln_b @ Wo.T
    extra = np.stack([wo.sum(axis=0), bo_eff]).astype(np.float32)
    bqs = bq * SCALE

    f16 = np.float16
    shared = {"wqk": wqk.astype(f16), "wv": wv_ext.astype(f16),
              "wo": wo.astype(f16), "extra": extra.astype(f16),
              "bqs": bqs, "bve": bve.astype(f16),
              "onesd": np.ones(128, f16)}
    in_maps = []
    for b in range(B):
        xT = np.ascontiguousarray(x[b].T).astype(f16)   # [C, N]
        in_maps.append({"xt": xT, **shared})
    return in_maps


_NC_CACHE = []


def _get_nc():
    if not _NC_CACHE:
        _NC_CACHE.append(build_kernel())
    return _NC_CACHE[0]


def kernel(**inputs) -> np.ndarray:
    nc = _get_nc()
    in_maps = prepare_in_maps(**inputs)
    res = run_bass_kernel_spmd(nc, in_maps, core_ids=list(range(B)))
    return np.stack([res.results[b]["y"] for b in range(B)], axis=0)


# revision 13
# speedup vs baseline: 1.4577x; 1.2294x over previous
"""Trainium2 Bass kernel for nn_Attention_28269474742408.

Single-layer attention block: qkv projections -> softmax attention ->
layernorm -> output projection, for x [8, 1024, 768] (B=8, N=1024, C=768,
H=12 heads, D=64).

Strategy: data parallel over the batch - one batch element per NeuronCore
(8 cores). Everything on-chip per core; no collectives.

Per-core structure (all channel-major, "T" = [channel, token]):
  - Host pre-transposes x[b] -> xT [768, 1024] and all weights -> W.T so
    projections/attention never need on-device transposes.
  - v is projected token-major [1024, 780]: 12 heads x (64 v-cols + a ones
    column); the ones column makes the PV matmul emit softmax denominators
    for free.  The v projection is emitted in column thirds interleaved
    into the pair-0 attention iterations so it hides under the exp stream.
  - Attention runs an mt-outer loop per head PAIR: the two heads' score
    matmuls (K=64 contraction) are row-tiled into the top/bottom halves of
    the PE array (tile_position via base_partition 0/64) so they run
    concurrently - the pair's scores cost one matmul slot instead of two.
  - softmax skips max-subtraction (scores bounded ~|3|, exp can't
    overflow); exp output feeds flash-style PV accumulation in PSUM.
  - Per-head softmax denominators: reciprocal is computed on a DMA-spread
    [128, 8] layout (DVE reciprocal is ~8 cyc per free-dim element, so a
    [1, 1024] row costs 6.5us but the spread costs ~60ns), then broadcast
    back over 64 partitions via a DRAM bounce.  The PSUM accumulator is
    freed early by copying it to SBUF before the division.
  - q/k projections for pair p+1 are emitted as filler units inside pair
    p's iterations, keeping TensorE dense while ScalarE runs exp.
  - LayerNorm is folded into the output projection: gamma/beta folded into
    Wo/bo on the host; mean/var via ones-matmuls, 4-way column-tiled into
    one PSUM bank (partition offsets 0/32/64/96 run concurrently);
    the -mean*colsum(Wo) + sqrt(var+eps)*bo rank-2 correction rides the
    output matmul as an extra K=2 accumulation; the rsqrt scale is applied
    per-token on eviction (built by DMA-spreading the std row to [128, 8]
    and one DVE reciprocal).
  - A dummy exp at t=0 preloads the activation table under the input DMA.
"""
import numpy as np

import concourse.bacc as bacc
import concourse.bass as bass
import concourse.tile as tile
from concourse import bass_isa
from concourse import mybir
from concourse.bass_utils import run_bass_kernel_spmd

F32 = mybir.dt.float32
F16 = mybir.dt.float16
AF = mybir.ActivationFunctionType
OP = mybir.AluOpType

B, N, C, H, D = 8, 1024, 768, 12, 64
KT = C // 128          # 6 channel tiles
NT = N // 128          # 8 token tiles
NP = H // 2            # 6 head pairs
VW = H * (D + 1)       # 780: v plus per-head ones column
VT = VW // 3           # 260: v projection column third
SCALE = D ** -0.5
EPS = 1e-5


def build_kernel():
    nc = bacc.Bacc("TRN2", target_bir_lowering=False)

    xt_d = nc.dram_tensor("xt", (C, N), F16, kind="ExternalInput")
    wv_d = nc.dram_tensor("wv", (C, VW), F16, kind="ExternalInput")
    wqk_d = nc.dram_tensor("wqk", (C, 2 * C), F16, kind="ExternalInput")
    wo_d = nc.dram_tensor("wo", (C, C), F16, kind="ExternalInput")
    extra_d = nc.dram_tensor("extra", (2, C), F16, kind="ExternalInput")
    bqs_d = nc.dram_tensor("bqs", (C,), F32, kind="ExternalInput")
    bve_d = nc.dram_tensor("bve", (VW,), F16, kind="ExternalInput")
    ones_d = nc.dram_tensor("onesd", (128,), F16, kind="ExternalInput")

    y_d = nc.dram_tensor("y", (N, C), F32, kind="ExternalOutput")
    rscr = nc.dram_tensor("rscr", (H, N), F32)     # internal: Z bounce
    rsci = nc.dram_tensor("rsci", (H, N), F32)     # internal: 1/Z bounce

    with tile.TileContext(nc) as tc:
        with tc.tile_pool(name="persist", bufs=1) as pp, \
             tc.tile_pool(name="attp", bufs=1) as attp, \
             tc.tile_pool(name="wop", bufs=1) as wop, \
             tc.tile_pool(name="sqp", bufs=1) as sqp:

            # ---- constants (loads emitted after the xT DMA, below) ----
            extra_t = pp.tile([2, C], F16, tag="extra", name="extra")
            bqs_t = [pp.tile([128, 1], F32, tag=f"bqs{m}", name=f"bqs{m}")
                     for m in range(NP)]
            ones_col = pp.tile([128, 1], F16, tag="ones_col", name="ones_col")
            eps_row = pp.tile([1, 1], F32, tag="eps_row", name="eps_row")
            warm_row = pp.tile([1, 1], F32, tag="warm_row", name="warm_row")
            nc.vector.memset(eps_row, EPS)
            # dummy exp: forces the exp act-table load under the input DMA
            nc.scalar.activation(out=warm_row, in_=eps_row, func=AF.Exp)
            # bias for v broadcast across all partitions (applied on evict)
            bve_bc = pp.tile([128, VW], F16, tag="bve_bc", name="bve_bc")

            att = [attp.tile([128, N], F16, tag=f"att{k}", name=f"att{k}")
                   for k in range(KT)]
            f_t = pp.tile([2, N], F16, tag="f_t", name="f_t")
            acol = pp.tile([128, NT], F32, tag="acol", name="acol")

            with tc.tile_pool(name="xtp", bufs=1) as xtp, \
                 tc.tile_pool(name="vp", bufs=1) as vp, \
                 tc.tile_pool(name="wvp", bufs=1) as wvp, \
                 tc.tile_pool(name="pairw", bufs=3) as pairw, \
                 tc.tile_pool(name="qkpair", bufs=3) as qkpair, \
                 tc.tile_pool(name="epool", bufs=4) as epool, \
                 tc.tile_pool(name="cpool", bufs=2) as cpool, \
                 tc.tile_pool(name="zspp", bufs=2) as zspp, \
                 tc.tile_pool(name="rbcp", bufs=2) as rbcp:

                xt = [xtp.tile([128, N], F16, tag=f"xt{k}", name=f"xt{k}")
                      for k in range(KT)]
                for k in range(KT):
                    nc.sync.dma_start(out=xt[k], in_=xt_d[k * 128:(k + 1) * 128, :])
                vt = [vp.tile([128, VW], F16, tag=f"vt{n}", name=f"vt{n}")
                      for n in range(NT)]
                wv_t = [wvp.tile([128, VW], F16, tag=f"wv{k}", name=f"wv{k}")
                        for k in range(KT)]
                wo_t = [wop.tile([128, C], F16, tag=f"wo{k}", name=f"wo{k}")
                        for k in range(KT)]
                sq = [sqp.tile([128, N], F16, tag=f"sq{k}", name=f"sq{k}")
                      for k in range(KT)]

                with tc.tile_pool(name="ps_s", bufs=2, space="PSUM") as ps_s, \
                     tc.tile_pool(name="ps_att", bufs=2, space="PSUM") as ps_att:
                    qk_tiles = {}

                    def load_pair_w(p):
                        pw = [pairw.tile([128, 256], F16, tag=f"pw{k}",
                                         name=f"pw{k}") for k in range(KT)]
                        for k in range(KT):
                            nc.sync.dma_start(
                                out=pw[k],
                                in_=wqk_d[k * 128:(k + 1) * 128,
                                          p * 256:(p + 1) * 256])
                        qt = qkpair.tile([128, N], F16, tag="qtp", name="qtp")
                        kt = qkpair.tile([128, N], F16, tag="ktp", name="ktp")
                        qk_tiles[p] = (pw, qt, kt)

                    def emit_proj_group(p, which, ch):
                        # one q-or-k projection chunk for pair p: [128, 512]
                        pw, qt, kt = qk_tiles[p]
                        off = 0 if which == "q" else 128
                        pg = ps_s.tile([128, 512], F32, tag="sp", name="pg")
                        for k in range(KT):
                            nc.tensor.matmul(
                                out=pg, lhsT=pw[k][:, off:off + 128],
                                rhs=xt[k][:, ch * 512:(ch + 1) * 512],
                                start=(k == 0), stop=(k == KT - 1),
                            )
                        if which == "q":
                            nc.vector.tensor_scalar(
                                out=qt[:, ch * 512:(ch + 1) * 512], in0=pg,
                                scalar1=bqs_t[p], scalar2=None, op0=OP.add,
                            )
                        else:
                            nc.vector.tensor_copy(
                                out=kt[:, ch * 512:(ch + 1) * 512], in_=pg)

                    def emit_v_unit(n, c0):
                        # one v-projection third: vt[n][:, c0:c0+VT]
                        pv = ps_s.tile([128, VT], F32, tag="sp", name="pv")
                        for k in range(KT):
                            nc.tensor.matmul(
                                out=pv,
                                lhsT=xt[k][:, n * 128:(n + 1) * 128],
                                rhs=wv_t[k][:, c0:c0 + VT],
                                start=(k == 0), stop=(k == KT - 1),
                            )
                        nc.vector.tensor_tensor(
                            out=vt[n][:, c0:c0 + VT], in0=pv,
                            in1=bve_bc[:, c0:c0 + VT], op=OP.add)

                    # ---- lead-in: pair-0 q/k projections ----
                    load_pair_w(0)
                    # small consts after the latency-critical xT/pw0 loads
                    for m in range(NP):
                        nc.sync.dma_start(
                            out=bqs_t[m],
                            in_=bqs_d[m * 128:(m + 1) * 128].unsqueeze(1))
                    nc.sync.dma_start(out=ones_col, in_=ones_d[:].unsqueeze(1))
                    nc.sync.dma_start(
                        out=bve_bc,
                        in_=bass.AP(tensor=bve_d[:].tensor, offset=0,
                                    ap=[[0, 128], [1, VW]]))
                    for k in range(KT):
                        nc.sync.dma_start(out=wv_t[k],
                                          in_=wv_d[k * 128:(k + 1) * 128, :])
                    for which, ch in (("q", 0), ("q", 1), ("k", 0), ("k", 1)):
                        emit_proj_group(0, which, ch)

                    # ---- filler queue: one unit per (pair, mt) iteration.
                    # v thirds: A (cols 0-259, pairs 0-1) during pair 0;
                    # B (260-519, pairs 2-3) during pair 1; C (520-779,
                    # pairs 4-5) during pair 2.  q/k projections for pair
                    # p+1 ride the first four iterations of pair p.
                    fillers = {}
                    for mt in range(NT):
                        fillers[(0, mt)] = [lambda n=mt: emit_v_unit(n, 0)]
                        fillers[(1, mt)] = [lambda n=mt: emit_v_unit(n, VT)]
                        fillers[(2, mt)] = [lambda n=mt: emit_v_unit(n, 2 * VT)]
                        fillers[(3, mt)] = []
                        fillers[(4, mt)] = []
                        fillers[(5, mt)] = []
                    for p in range(NP - 1):
                        for i, (which, ch) in enumerate(
                                (("q", 0), ("q", 1), ("k", 0), ("k", 1))):
                            fillers[(p, i)].append(
                                lambda p=p, w=which, c=ch:
                                    emit_proj_group(p + 1, w, c))

                    # ---- attention: flat software-pipelined loop ----
                    # Per iteration g=(p,mt): fillers, QK, exp, then the PV
                    # of the PREVIOUS iteration (so PV never sits between
                    # exp(h1,i) and the next QK on the critical path).
                    # Each pair's denominator chain (copy -> spread ->
                    # reciprocal -> broadcast) is emitted at the next pair
                    # boundary, and its divides one pair later still, so
                    # the ~6us DMA round-trip never blocks the DVE FIFO.
                    pa_map, ev_map = {}, {}

                    def evict_block(p):
                        ev = []
                        for hh in range(2):
                            h = 2 * p + hh
                            cp = cpool.tile([65, N], F32, tag="cp", name="cp")
                            nc.vector.tensor_copy(out=cp, in_=pa_map[p][hh])
                            nc.gpsimd.dma_start(out=rscr[h:h + 1, :],
                                              in_=cp[64:65, :])
                            # spread Z as [32, 32]: 32 contiguous 128B runs
                            # (a [128, 8] spread would be 1024 tiny DMA
                            # descriptors and takes several us per hop)
                            zsp = zspp.tile([32, 32], F32, tag="zsp",
                                            name="zsp")
                            src = rscr[h:h + 1, :]
                            nc.gpsimd.dma_start(
                                out=zsp,
                                in_=bass.AP(tensor=src.tensor, offset=src.offset,
                                            ap=[[32, 32], [1, 32]]))
                            zspi = zspp.tile([32, 32], F32, tag="zspi",
                                             name="zspi")
                            nc.vector.reciprocal(out=zspi, in_=zsp)
                            dst = rsci[h:h + 1, :]
                            nc.gpsimd.dma_start(
                                out=bass.AP(tensor=dst.tensor, offset=dst.offset,
                                            ap=[[32, 32], [1, 32]]),
                                in_=zspi)
                            rbc = rbcp.tile([64, N], F32, tag="rbc", name="rbc")
                            nc.gpsimd.dma_start(
                                out=rbc,
                                in_=bass.AP(tensor=dst.tensor, offset=dst.offset,
                                            ap=[[0, 64], [1, N]]))
                            ev.append((cp, rbc))
                        ev_map[p] = ev

                    def div_block(p):
                        for hh in range(2):
                            cp, rbc = ev_map[p][hh]
                            nc.vector.tensor_tensor(
                                out=att[p][hh * 64:hh * 64 + 64, :],
                                in0=cp[0:64, :], in1=rbc, op=OP.mult)
                        del ev_map[p]
                        nc.vector.tensor_tensor(out=sq[p], in0=att[p],
                                                in1=att[p], op=OP.mult)

                    def emit_pv(p, mt, e_t):
                        if mt == 0:
                            pa_map[p] = [ps_att.tile([65, N], F32, tag="pa",
                                                     name=f"pa{hh}")
                                         for hh in range(2)]
                        for hh in range(2):
                            h = 2 * p + hh
                            for ch in range(2):
                                nc.tensor.matmul(
                                    out=pa_map[p][hh][:, ch * 512:(ch + 1) * 512],
                                    lhsT=vt[mt][:, h * 65:(h + 1) * 65],
                                    rhs=e_t[hh][:, ch * 512:(ch + 1) * 512],
                                    start=(mt == 0), stop=(mt == NT - 1),
                                )

                    prev = None
                    for g in range(NP * NT):
                        p, mt = divmod(g, NT)
                        if mt == 0:
                            if p + 1 < NP:
                                load_pair_w(p + 1)
                            if p == 2:
                                # wo needed only by the output projection
                                for k in range(KT):
                                    nc.sync.dma_start(
                                        out=wo_t[k],
                                        in_=wo_d[k * 128:(k + 1) * 128, :])
                                nc.sync.dma_start(out=extra_t, in_=extra_d[:, :])
                        _, qt, kt = qk_tiles[p]
                        for fn in fillers.pop((p, mt), ()):
                            fn()
                        s_t = [ps_s.tile([128, N], F32, tag="sp",
                                         name=f"s{hh}") for hh in range(2)]
                        # row-tiled pair scores: head hh uses PE array
                        # rows 64*hh..64*hh+63 - both run concurrently
                        for ch in range(2):
                            for hh in range(2):
                                hr = hh * 64
                                nc.tensor.matmul(
                                    out=s_t[hh][:, ch * 512:(ch + 1) * 512],
                                    lhsT=kt[hr:hr + 64, mt * 128:(mt + 1) * 128],
                                    rhs=qt[hr:hr + 64, ch * 512:(ch + 1) * 512],
                                    start=True, stop=True,
                                )
                        e_t = []
                        for hh in range(2):
                            e = epool.tile([128, N], F16, tag="e", name="e")
                            nc.scalar.activation(out=e, in_=s_t[hh], func=AF.Exp)
                            e_t.append(e)
                        if prev is not None:
                            pp, pmt, pe = prev
                            emit_pv(pp, pmt, pe)
                            if pmt == NT - 1:
                                evict_block(pp)
                                if pp >= 1:
                                    div_block(pp - 1)
                        prev = (p, mt, e_t)
                    pp, pmt, pe = prev
                    emit_pv(pp, pmt, pe)
                    evict_block(pp)
                    div_block(pp - 1)
                    div_block(pp)

            # ---- phase C: LN stats + output projection ----
            with tc.tile_pool(name="rowpool", bufs=1) as rowpool, \
                 tc.tile_pool(name="ypool", bufs=3) as ypool:

                with tc.tile_pool(name="ps_row", bufs=1, space="PSUM") as ps_row:
                    # 4-way column-tiled stats in one PSUM bank: rows
                    # 0/32/64/96 hold sum(att) ch0/ch1, sum(att^2) ch0/ch1
                    rows = ps_row.tile([97, 512], F32, tag="rows", name="rows")
                    for k in range(KT):
                        for j, (srcs, ch) in enumerate(
                                ((att, 0), (att, 1), (sq, 0), (sq, 1))):
                            nc.tensor.matmul(
                                out=rows[32 * j:32 * j + 1, :],
                                lhsT=ones_col,
                                rhs=srcs[k][:, ch * 512:(ch + 1) * 512],
                                start=(k == 0), stop=(k == KT - 1),
                                tile_position=(0, 32 * j),
                            )
                    mrow = rowpool.tile([1, N], F32, tag="mrow", name="mrow")
                    t0 = rowpool.tile([1, N], F32, tag="t0", name="t0")
                    for ch in range(2):
                        sl = slice(ch * 512, (ch + 1) * 512)
                        nc.scalar.mul(out=mrow[:, sl],
                                      in_=rows[32 * ch:32 * ch + 1, :],
                                      mul=1.0 / C)
                        nc.scalar.mul(out=f_t[0:1, sl],
                                      in_=rows[32 * ch:32 * ch + 1, :],
                                      mul=-1.0 / C)
                        nc.scalar.mul(out=t0[:, sl],
                                      in_=rows[64 + 32 * ch:64 + 32 * ch + 1, :],
                                      mul=1.0 / C)
                    m2 = rowpool.tile([1, N], F32, tag="m2", name="m2")
                    nc.vector.tensor_tensor(out=m2, in0=mrow, in1=mrow, op=OP.mult)
                    varr = rowpool.tile([1, N], F32, tag="varr", name="varr")
                    nc.vector.tensor_tensor(out=varr, in0=t0, in1=m2, op=OP.subtract)
                    stdrow = rowpool.tile([1, N], F16, tag="stdrow", name="stdrow")
                    nc.scalar.activation(out=stdrow, in_=varr, func=AF.Sqrt,
                                         bias=eps_row, scale=1.0)
                    # DMA (partition-unconstrained) assembles row 1 of f_t
                    nc.sync.dma_start(out=f_t[1:2, :], in_=stdrow)
                    # per-token 1/std scale [128, NT]: transpose the std row
                    # via NT tiny K=1 matmuls, then one PSUM reciprocal -
                    # zero DMA hops on the output-projection critical path
                    with tc.tile_pool(name="ps_pt", bufs=1,
                                      space="PSUM") as ps_pt:
                        pt = ps_pt.tile([128, NT], F32, tag="pt", name="pt")
                        for j in range(NT):
                            nc.tensor.matmul(
                                out=pt[:, j:j + 1],
                                lhsT=stdrow[0:1, j * 128:(j + 1) * 128],
                                rhs=ones_col[0:1, 0:1],
                                start=True, stop=True)
                        nc.vector.reciprocal(out=acol, in_=pt)

                with tc.tile_pool(name="ps_out", bufs=3, space="PSUM") as ps_out:
                    for n in range(NT):
                        po = ps_out.tile([128, C], F32, tag="po", name="po")
                        for c0, cw in ((0, 512), (512, C - 512)):
                            for k in range(KT):
                                nc.tensor.matmul(
                                    out=po[:, c0:c0 + cw],
                                    lhsT=att[k][:, n * 128:(n + 1) * 128],
                                    rhs=wo_t[k][:, c0:c0 + cw],
                                    start=(k == 0), stop=False,
                                )
                            nc.tensor.matmul(
                                out=po[:, c0:c0 + cw],
                                lhsT=f_t[:, n * 128:(n + 1) * 128],
                                rhs=extra_t[:, c0:c0 + cw],
                                start=False, stop=True,
                            )
                        yt = ypool.tile([128, C], F32, tag="yt", name="yt")
                        nc.vector.tensor_scalar(
                            out=yt, in0=po, scalar1=acol[:, n:n + 1], scalar2=None,
                            op0=OP.mult)
                        nc.sync.dma_start(out=y_d[n * 128:(n + 1) * 128, :], in_=yt)

    nc.compile()
    return nc


def prepare_in_maps(x, Wq, bq, Wk, bk, Wv, bv, Wo, bo, ln_g, ln_b):
    x = np.asarray(x, np.float32)
    Wq = np.asarray(Wq, np.float32); bq = np.asarray(bq, np.float32)
    Wk = np.asarray(Wk, np.float32)
    Wv = np.asarray(Wv, np.float32); bv = np.asarray(bv, np.float32)
    Wo = np.asarray(Wo, np.float32); bo = np.asarray(bo, np.float32)
    ln_g = np.asarray(ln_g, np.float32); ln_b = np.asarray(ln_b, np.float32)

    wq = np.ascontiguousarray(Wq.T) * SCALE
    wk = np.ascontiguousarray(Wk.T)
    wv = np.ascontiguousarray(Wv.T)            # [C, C]
    wv_ext = np.zeros((C, VW), np.float32)
    bve = np.zeros((VW,), np.float32)
    for h in range(H):
        wv_ext[:, h * 65: h * 65 + 64] = wv[:, h * 64:(h + 1) * 64]
        bve[h * 65: h * 65 + 64] = bv[h * 64:(h + 1) * 64]
        bve[h * 65 + 64] = 1.0                 # ones column for denominators
    # pair-blocked q/k weights: [wq_p | wk_p] per 128-channel head pair
    wqk = np.zeros((C, 2 * C), np.float32)
    for p in range(NP):
        wqk[:, p * 256: p * 256 + 128] = wq[:, p * 128:(p + 1) * 128]
        wqk[:, p * 256 + 128:(p + 1) * 256] = wk[:, p * 128:(p + 1) * 128]
    wo = ln_g[:, None] * np.ascontiguousarray(Wo.T)
    bo_eff = bo + ln_b @ Wo.T
    extra = np.stack([wo.sum(axis=0), bo_eff]).astype(np.float32)
    bqs = bq * SCALE

    f16 = np.float16
    shared = {"wqk": wqk.astype(f16), "wv": wv_ext.astype(f16),
              "wo": wo.astype(f16), "extra": extra.astype(f16),
              "bqs": bqs, "bve": bve.astype(f16),
              "onesd": np.ones(128, f16)}
    in_maps = []
    for b in range(B):
        xT = np.ascontiguousarray(x[b].T).astype(f16)   # [C, N]
        in_maps.append({"xt": xT, **shared})
    return in_maps


_NC_CACHE = []


def _get_nc():
    if not _NC_CACHE:
        _NC_CACHE.append(build_kernel())
    return _NC_CACHE[0]


def kernel(**inputs) -> np.ndarray:
    nc = _get_nc()
    in_maps = prepare_in_maps(**inputs)
    res = run_bass_kernel_spmd(nc, in_maps, core_ids=list(range(B)))
    return np.stack([res.results[b]["y"] for b in range(B)], axis=0)
